# revision 41
# baseline (speedup 1.0000x reference)
"""Causal attention (RoPE, 16 heads, L=2048, H=2048) on 8 trn2 NeuronCores.

Sharding: tensor-parallel over heads. Core i handles heads 2i, 2i+1
(d=128 each): column-parallel q/k/v projections, row-parallel o_proj,
host-side sum of the 8 partial outputs.

Single fused instruction stream; block i interleaves projection
chunk lc=i+1 with attention q-chunk qc=i and o_proj of qc=i-1 so the
ACT-bound exp work hides under PE-bound projection matmuls.
  - Everything bf16 except PSUM accumulation and the RoPE psum reads
    (fp32r matmuls measure ~30% slower than bf16, so bf16/fp16
    wherever numerics allow).
  - Q^T/K^T in [d, L] layout (weight-stationary, N=512); RoPE on DVE
    reading PSUM directly, bf16 temporaries.  V in natural [L, d]
    layout (x-stationary, N=256) - no transposes.
  - Causal mask: S computed on valid columns [128j:512] of diagonal
    tiles; exp, PV and the ptsum adds all sliced to the same range so
    no memset of dead columns is needed; the diagonal 128-col block
    of pt is multiplied by one shared [128,128] bf16 triangle on DVE.
  - Softmax denominators: DVE/gpsimd accumulate exp tiles elementwise
    into fp16 ptsum (2x DVE rate vs f32); one fp16 ones-matmul per
    (head, q-chunk) reduces over partitions; DVE reciprocal +
    multiply normalizes into bf16 ot.
  - x host-rearranged into wave-major [lc, 128, kc, 512] so the DMA
    arrival order exactly matches the chains' (lc, kc) consumption
    order with large contiguous lines; weights partition-major.
  - o_proj per 128-row q-tile into a contiguous [128, 2048] bf16
    buffer; last q-chunk's evictions rotate over ACT/DVE/Pool and its
    output DMAs go out per-512-col slice on three queues to shorten
    the tail (partials summed on host in f64).
"""
import numpy as np

L = 2048
H = 2048
NH = 16
D = 128          # head dim
NCORES = 8
HPC = NH // NCORES   # heads per core = 2
ROPE_BASE = 10000.0
KC = H // 128        # 16 contraction chunks
LCN = 4              # L chunks of 512
QCN = 4              # q chunks of 512

_CACHE = {}


def _rope_tables():
    inv_freq = 1.0 / (ROPE_BASE ** (np.arange(0, D, 2, dtype=np.float32) / D))
    t = np.arange(L, dtype=np.float32)
    freqs = np.outer(t, inv_freq).astype(np.float32)          # [L, D/2]
    emb = np.concatenate([freqs, freqs], axis=-1)             # [L, D]
    cos = np.cos(emb).astype(np.float32)                      # [L, D]
    sin = np.sin(emb).astype(np.float32)
    cosT = np.ascontiguousarray(cos.T)                        # [D, L]
    sinT = np.ascontiguousarray(sin.T)
    sinTs = sinT.copy()
    sinTs[: D // 2] = -sinT[: D // 2]                         # sign-folded
    # partition-swapped so DVE operand base partitions match:
    # sinsw[p] = sinTs[(p+64) % 128]
    sinsw = np.concatenate([sinTs[D // 2:], sinTs[: D // 2]], axis=0)
    return cosT, np.ascontiguousarray(sinsw)


def _causal_masks():
    # multiplicative triangle for the diagonal 128x128 block (same for
    # every j): keep (1.0) iff col >= row
    row = np.arange(128)[:, None]
    col = np.arange(128)[None, :]
    return np.where(col - row >= 0, 1.0, 0.0).astype(np.float32)


def _build_nc():
    import concourse.bacc as bacc
    import concourse.mybir as mybir
    from concourse import tile
    from contextlib import ExitStack

    f32 = mybir.dt.float32
    f16 = mybir.dt.float16
    bf16 = mybir.dt.bfloat16
    AF = mybir.ActivationFunctionType
    OP = mybir.AluOpType

    nc = bacc.Bacc("TRN2", target_bir_lowering=False, debug=False)

    # weights host-rearranged to partition-major so DMAs are contiguous;
    # x wave-major: [lc, 128, kc, 512] so arrival order == consumption
    xW_d = nc.dram_tensor("xW", (LCN, 128, KC, 512), bf16, kind="ExternalInput")
    # wq|wk fused per kc so weight DMAs have 4KB partition lines
    wqk_d = nc.dram_tensor("wqkT", (128, KC, 512), bf16, kind="ExternalInput")
    wv_d = nc.dram_tensor("wvT", (128, KC, HPC * D), bf16, kind="ExternalInput")
    wo_d = nc.dram_tensor("woP", (128, HPC, H), bf16, kind="ExternalInput")
    cos_d = nc.dram_tensor("cosT", (D, L), bf16, kind="ExternalInput")
    sin_d = nc.dram_tensor("sinTs", (D, L), bf16, kind="ExternalInput")
    msk_d = nc.dram_tensor("masks", (128, 128), bf16, kind="ExternalInput")
    ones_d = nc.dram_tensor("ones", (128, 128), f16, kind="ExternalInput")
    out_d = nc.dram_tensor("out", (L, H), bf16, kind="ExternalOutput")

    with tile.TileContext(nc) as tc, ExitStack() as top:
        per = top.enter_context(tc.tile_pool(name="per", bufs=1))

        wqk_sb = per.tile([128, KC, 512], bf16)
        wv_sb = per.tile([128, KC, HPC * D], bf16)
        wo_sb = per.tile([128, HPC, H], bf16)
        cos_sb = per.tile([128, L], bf16)
        sin_sb = per.tile([128, L], bf16)
        msk_sb = per.tile([128, 128], bf16)
        ones_sb = per.tile([128, 128], f16)
        qt_sb = [[per.tile([128, 512], bf16, name=f"qt{h}_{c}")
                  for c in range(QCN)] for h in range(HPC)]
        # normalized O^T in bf16 (o_proj runs fully bf16)
        ot_sb = [[per.tile([128, 512], bf16, name=f"ot{h}_{c}")
                  for c in range(QCN)] for h in range(HPC)]
        kt_sb = [[per.tile([128, 512], bf16, name=f"kt{h}_{c}")
                  for c in range(LCN)] for h in range(HPC)]
        # natural V: [l within 128-tile, lt, d of both heads]
        v_sb = per.tile([128, KC, HPC * D], bf16, name="v")
        ptsum = [per.tile([128, 512], f16, name=f"ptsum{h}") for h in range(HPC)]
        # all of x^T stays resident, wave-major: xw[lc][:, kc, :] is the
        # [128, 512] moving block for chain (lc, kc); per-wave DRAM is
        # contiguous (16KB partition lines) so DMA runs at full rate
        xw = [per.tile([128, KC, 512], bf16, name=f"xw{w}") for w in range(LCN)]

        ptpool = top.enter_context(tc.tile_pool(name="pt", bufs=4))
        tpool = top.enter_context(tc.tile_pool(name="tmp", bufs=1))
        obpool = top.enter_context(tc.tile_pool(name="ob", bufs=2))
        psum_pools = {}

        # ---------- PE warm-up ----------
        # The PE p-state ramps with continuous activity (cold matmuls run
        # ~2-4x slower).  Burn dummy matmuls on a memset tile during the
        # initial DMA wait so the first real matmul runs at full clock.
        warm_sb = per.tile([128, 128], bf16, name="warm")
        from contextlib import ExitStack as _ES
        with _ES() as warm_scope:
            wp = warm_scope.enter_context(
                tc.tile_pool(name="warm", bufs=1, space="PSUM"))
            nc.vector.memset(warm_sb[:], 0.0)
            wps = wp.tile([128, 256], f32, tag="w")
            for _ in range(44):
                nc.tensor.matmul(wps[:, 0:128], warm_sb[:], warm_sb[:],
                                 start=True, stop=True)

        # ---------- initial DMAs ----------
        # Rings serialize per transfer at a line-size-dependent rate
        # (1KB lines ~42B/ns, 4KB ~90B/ns; gpsimd ~1.5x), so everything
        # early moves in 4-kc groups with 4KB partition lines, braided
        # so each ring's arrival order matches lc0's (kc-ascending)
        # consumption: gpsimd carries xw0 alone, sync/scalar alternate
        # wqk groups then split wv.
        def emit_xw(eng, w, a, b):
            eng.dma_start(xw[w][:, a:b, :], xW_d[w, :, a:b, :])

        def emit_wqk(eng, a, b):
            eng.dma_start(wqk_sb[:, a:b, :], wqk_d[:, a:b, :])

        nc.scalar.dma_start(msk_sb[:], msk_d[:])
        nc.scalar.dma_start(ones_sb[:], ones_d[:])
        # per-kc-pair braid: fused wqk pairs (2KB lines) alternate
        # sync/scalar, x pairs stream on gpsimd; arrivals track the
        # interleaved per-kc consumption of the lc0 chains
        for p in range(8):
            emit_xw(nc.gpsimd, 0, 2 * p, 2 * p + 2)
            emit_wqk(nc.sync if p % 2 == 0 else nc.scalar,
                     2 * p, 2 * p + 2)
        nc.scalar.dma_start(cos_sb[:, 0:512], cos_d[:, 0:512])
        nc.scalar.dma_start(sin_sb[:, 0:512], sin_d[:, 0:512])
        nc.gpsimd.dma_start(wv_sb[:, 8:16, :], wv_d[:, 8:16, :])
        nc.sync.dma_start(wv_sb[:, 0:8, :], wv_d[:, 0:8, :])

        # ---------------- thunk builders ----------------
        def rope_evict(ps, dst, lc):
            cs = slice(lc * 512, (lc + 1) * 512)
            t1 = tpool.tile([128, 512], bf16, tag="t1")
            t2 = tpool.tile([128, 512], bf16, tag="t2")
            nc.vector.tensor_tensor(t2[:], ps[:], cos_sb[:, cs], OP.mult)
            nc.vector.tensor_tensor(
                t1[0:64, :], ps[64:128, :], sin_sb[64:128, cs], OP.mult)
            nc.vector.tensor_tensor(
                t1[64:128, :], ps[0:64, :], sin_sb[0:64, cs], OP.mult)
            nc.vector.tensor_tensor(dst[:], t1[:], t2[:], OP.add)

        def qk_chain(pool, tag, off, lc, h, dst):
            """16 matmul thunks accumulating one head's Q^T/K^T chunk.

            off: 0 for Q, HPC*D for K (column offset in the fused wqk)."""
            ps = pool.tile([128, 512], f32, tag=tag, name="ps")
            thunks = []
            for kc in range(KC):
                def mm(kc=kc, ps=ps, off=off, h=h, lc=lc):
                    c = off + h * D
                    nc.tensor.matmul(
                        ps[:], wqk_sb[:, kc, c:c + D],
                        xw[lc][:, kc, :],
                        start=(kc == 0), stop=(kc == KC - 1))
                thunks.append(mm)
            thunks[-1] = (thunks[-1], lambda ps=ps, dst=dst, lc=lc:
                          rope_evict(ps, dst, lc))
            return thunks

        def v_chain(pool, tag, lc, lt):
            ps = pool.tile([128, 512], f32, tag=tag, name="ps")
            thunks = []
            for kc in range(KC):
                def mm(kc=kc, ps=ps, lt=lt, lc=lc):
                    nc.tensor.matmul(
                        ps[:, 0:HPC * D],
                        xw[lc][:, kc, lt * 128:(lt + 1) * 128],
                        wv_sb[:, kc, :],
                        start=(kc == 0), stop=(kc == KC - 1))
                thunks.append(mm)
            def ev(ps=ps, lt=lt, lc=lc):
                nc.scalar.copy(v_sb[:, lc * 4 + lt, :], ps[:, 0:HPC * D])
            thunks[-1] = (thunks[-1], ev)
            return thunks

        def proj_thunks(lc):
            """128 PE thunks for chunk lc: Q0,K0,Q1,K1 then V (2 psum bufs).

            Q/K alternate so each chain's RoPE eviction has a full chain of
            slack before its psum slot is reused."""
            pj = psum_pools["pj"]
            thunks = []
            thunks += qk_chain(pj, "pj", 0, lc, 0, qt_sb[0][lc])
            thunks += qk_chain(pj, "pj", HPC * D, lc, 0, kt_sb[0][lc])
            thunks += qk_chain(pj, "pj", 0, lc, 1, qt_sb[1][lc])
            thunks += qk_chain(pj, "pj", HPC * D, lc, 1, kt_sb[1][lc])
            for lt in range(4):
                thunks += v_chain(pj, "pj", lc, lt)
            return thunks

        def att_units(qc):
            """n_kt+2 units: pipelined S/exp then PV/accum one kt behind."""
            n_kt = 4 * qc + 4
            units = []
            pts = {}    # (kt, h) -> (pt tile, c0)
            acc_p = psum_pools["acc"]
            sps_p = psum_pools["sps"]
            sums = [acc_p.tile([128, 512], f32, tag=f"o{h}", name=f"ops{h}")
                    for h in range(HPC)]

            def step(u, qc=qc, n_kt=n_kt):
                if u < n_kt:
                    kt = u
                    diag = kt >= 4 * qc
                    # diagonal tiles: valid cols >= 128j.  S, exp, PV and
                    # the ptsum add are all sliced to [128j:512]; the
                    # triangle block [128j:128j+128) is masked on DVE.
                    j = kt - 4 * qc
                    c0 = 128 * j if diag else 0
                    for h in range(HPC):
                        # qc=3 runs with no proj work interleaved, so the
                        # idle pj pool lends 2 more S banks: 4-deep S/exp
                        # pipelining instead of 2-deep.
                        sp = psum_pools["pj"] if (qc == QCN - 1 and u % 2) \
                            else sps_p
                        s = sp.tile([128, 512], f32, tag="s" if sp is sps_p
                                    else "pj")
                        nc.tensor.matmul(
                            s[:, c0:],
                            kt_sb[h][kt // 4][:, (kt % 4) * 128:(kt % 4 + 1) * 128],
                            qt_sb[h][qc][:, c0:],
                            start=True, stop=True)
                        pt = ptpool.tile([128, 512], bf16, tag=f"pt{h}")
                        nc.scalar.activation(pt[:, c0:], s[:, c0:], AF.Exp)
                        if diag:
                            tb = slice(128 * j, 128 * (j + 1))
                            # final block is DVE/ACT-bound: masks -> Pool
                            meng = nc.gpsimd if qc == QCN - 1 else nc.vector
                            meng.tensor_tensor(
                                pt[:, tb], pt[:, tb], msk_sb[:], OP.mult)
                        pts[(kt, h)] = (pt, c0)
                if u >= 1:
                    kt = u - 1
                    for h in range(HPC):
                        pt, c0 = pts[(kt, h)]
                        nc.tensor.matmul(
                            sums[h][:, c0:], v_sb[:, kt, h * D:(h + 1) * D],
                            pt[:, c0:],
                            start=(kt == 0), stop=(kt == n_kt - 1),
                            skip_group_check=bool(c0))
                    for h in range(HPC):
                        # h0 on DVE, h1 on gpsimd; last kt both on DVE (it
                        # sits on the flush critical path and DVE is faster)
                        pt, c0 = pts[(kt, h)]
                        eng = nc.vector if (h == 0 or kt == n_kt - 1) \
                            else nc.gpsimd
                        if kt == 0:
                            eng.tensor_copy(ptsum[h][:], pt[:])
                        else:
                            eng.tensor_tensor(
                                ptsum[h][:, c0:], ptsum[h][:, c0:],
                                pt[:, c0:], OP.add)
                        del pts[(kt, h)]

            def flush(qc=qc):
                for h in range(HPC):
                    den = sps_p.tile([128, 512], f32, tag="s")
                    nc.tensor.matmul(den[:], ones_sb[:], ptsum[h][:],
                                     start=True, stop=True)
                    rc = tpool.tile([128, 512], f32, tag="rc")
                    # last chunk sliced so o_proj can start on slice 0 early
                    nsl = 4 if qc == QCN - 1 else 1
                    for sl in range(nsl):
                        c = slice(sl * 512 // nsl, (sl + 1) * 512 // nsl)
                        nc.vector.reciprocal_approx_fast(rc[:, c], den[:, c])
                        nc.vector.tensor_tensor(ot_sb[h][qc][:, c],
                                                sums[h][:, c], rc[:, c],
                                                OP.mult)

            for u in range(n_kt + 1):
                units.append(lambda u=u: step(u))
            units.append(flush)
            return units

        ob_tiles = {}

        def oproj_units(qc, tailq=False):
            """16 units: (qt4, hcn) -> 2 matmuls + evict; DMA per qt4.

            For the tail chunk the evictions rotate over ACT/DVE/Pool and
            each 512-col slice DMAs out on its own queue as soon as its
            eviction lands, so the post-matmul tail is one slice deep."""
            units = []
            for qt4 in range(4):
                for hcn in range(4):
                    def grp(qc=qc, qt4=qt4, hcn=hcn, tailq=tailq):
                        if hcn == 0:
                            ob_tiles[(qc, qt4)] = obpool.tile(
                                [128, H], bf16, tag="ob", name="ob")
                        ob = ob_tiles[(qc, qt4)]
                        if tailq:
                            # nothing else owns PSUM in the tail: rotate po
                            # over 6 banks (pop+pj+sps) so evictions never
                            # stall the PE
                            pnm = ["pop", "pj", "sps"][(qt4 * 4 + hcn) % 3]
                            ptag = {"pop": "po", "pj": "pj", "sps": "s"}[pnm]
                            po = psum_pools[pnm].tile([128, 512], f32,
                                                      tag=ptag, name="po")
                        else:
                            po = psum_pools["pop"].tile([128, 512], f32,
                                                        tag="po", name="po")
                        for h in range(HPC):
                            nc.tensor.matmul(
                                po[:], ot_sb[h][qc][:, qt4 * 128:(qt4 + 1) * 128],
                                wo_sb[:, h, hcn * 512:(hcn + 1) * 512],
                                start=(h == 0), stop=(h == HPC - 1))
                        dst = ob[:, hcn * 512:(hcn + 1) * 512]
                        qt = qc * 4 + qt4
                        cs = slice(hcn * 512, (hcn + 1) * 512)
                        if qc == 2:
                            # merged into the ACT-bound final att block:
                            # keep ACT free for exp, evict on DVE only
                            nc.vector.tensor_copy(dst, po[:])
                        elif hcn % 2 == 0:
                            nc.scalar.copy(dst, po[:])
                        else:
                            nc.vector.tensor_copy(dst, po[:])
                        if tailq:
                            # DMA each 512-col slice as two 64-row halves
                            # on separate rings: a [64,512] transfer is
                            # descriptor-bound (~1.5us) so halves in
                            # parallel keep the drain off the critical tail
                            u = qt4 * 4 + hcn
                            if qt4 == 3:
                                # last tile: HWDGE rings only (the gpsimd
                                # software-DGE trigger adds ~1us latency)
                                qa, qb = nc.sync, nc.scalar
                            else:
                                qa = [nc.sync, nc.gpsimd, nc.scalar][u % 3]
                                qb = [nc.sync, nc.gpsimd, nc.scalar][
                                    (u + 1) % 3]
                            r0 = slice(qt * 128, qt * 128 + 64)
                            r1 = slice(qt * 128 + 64, (qt + 1) * 128)
                            qa.dma_start(out_d[r0, cs], ob[0:64, cs])
                            qb.dma_start(out_d[r1, cs], ob[64:128, cs])
                            if hcn == 3:
                                del ob_tiles[(qc, qt4)]
                        else:
                            # mid-kernel: per-hcn-pair [128,1024] slices on
                            # alternating rings (2KB lines); qc=2 avoids
                            # gpsimd (its engine is busy with ptsum adds)
                            ring = ([nc.sync, nc.scalar] if qc == 2
                                    else [nc.sync, nc.gpsimd])
                            if hcn == 1:
                                eng = ring[qt4 % 2]
                                eng.dma_start(out_d[qt * 128:(qt + 1) * 128,
                                                    0:1024], ob[:, 0:1024])
                            elif hcn == 3:
                                eng = ring[(qt4 + 1) % 2]
                                eng.dma_start(out_d[qt * 128:(qt + 1) * 128,
                                                    1024:2048],
                                              ob[:, 1024:2048])
                                del ob_tiles[(qc, qt4)]
                    units.append(grp)
            return units

        def emit_interleaved(pe_thunks, unit_list, lead=12):
            """Spread units evenly among the PE thunk stream.

            lead: PE thunks emitted before the first unit — the PE queue
            is in-order, so a unit stalled on a cross-engine dependency
            (e.g. S waiting on RoPE) must not block the whole stream."""
            n_t, n_u = len(pe_thunks), len(unit_list)
            span = max(1, n_t - lead + 1)
            ui = 0
            for i, th in enumerate(pe_thunks):
                while ui < n_u and ui * span <= max(0, i - lead) * n_u \
                        and i >= lead:
                    unit_list[ui]()
                    ui += 1
                if isinstance(th, tuple):
                    th[0]()
                    th[1]()
                else:
                    th()
            while ui < n_u:
                unit_list[ui]()
                ui += 1

        # ---------------- emission ----------------
        # Post-lc0 x waves and small tensors, deadline-ordered.  xw1 is
        # the first post-lc0 dependency, so its head is hedged across all
        # three rings; cos/sin feed the RoPE evictions that hold proj
        # psum banks, so each wave's slice streams just ahead of its use.
        emit_xw(nc.scalar, 1, 0, 2)
        emit_xw(nc.sync, 1, 12, 14)
        emit_xw(nc.sync, 1, 14, 16)
        nc.sync.dma_start(cos_sb[:, 512:1024], cos_d[:, 512:1024])
        nc.sync.dma_start(sin_sb[:, 512:1024], sin_d[:, 512:1024])
        emit_xw(nc.gpsimd, 1, 2, 4)
        emit_xw(nc.gpsimd, 1, 4, 8)
        emit_xw(nc.gpsimd, 1, 8, 12)
        nc.scalar.dma_start(wo_sb[:], wo_d[:])
        nc.sync.dma_start(cos_sb[:, 1024:2048], cos_d[:, 1024:2048])
        nc.sync.dma_start(sin_sb[:, 1024:2048], sin_d[:, 1024:2048])
        emit_xw(nc.gpsimd, 2, 0, 8)
        emit_xw(nc.sync, 2, 8, 16)
        emit_xw(nc.gpsimd, 3, 0, 8)
        emit_xw(nc.sync, 3, 8, 16)

        # lc0 in its own wide psum pool (closed before the steady-state
        # pools open): Q0/Q1/K0/K1 interleaved per kc, then V lt-major.
        with ExitStack() as lc0_scope:
            lc0_p = lc0_scope.enter_context(
                tc.tile_pool(name="lc0", bufs=8, space="PSUM"))
            qk = [(0, 0, qt_sb[0][0]), (0, 1, qt_sb[1][0]),
                  (HPC * D, 0, kt_sb[0][0]), (HPC * D, 1, kt_sb[1][0])]
            chains = [qk_chain(lc0_p, "l0", off, 0, h, dst)
                      for off, h, dst in qk]
            for kc in range(KC):
                for ch in chains:
                    th = ch[kc]
                    if isinstance(th, tuple):
                        th[0]()
                        th[1]()
                    else:
                        th()
            for lt in range(4):
                for th in v_chain(lc0_p, "l0", 0, lt):
                    if isinstance(th, tuple):
                        th[0]()
                        th[1]()
                    else:
                        th()

        psum_pools["pj"] = top.enter_context(
            tc.tile_pool(name="pj", bufs=2, space="PSUM"))
        psum_pools["sps"] = top.enter_context(
            tc.tile_pool(name="sps", bufs=2, space="PSUM"))
        psum_pools["acc"] = top.enter_context(
            tc.tile_pool(name="acc", bufs=1, space="PSUM"))
        psum_pools["pop"] = top.enter_context(
            tc.tile_pool(name="pop", bufs=2, space="PSUM"))

        for i in range(QCN):
            units = att_units(i)
            if i == 0:
                # att(0)'s first steps consume only lc0 results: emit
                # them as PE filler before proj(1), stretching xw1's
                # arrival deadline past the early DMA-bandwidth crunch
                for u in units[:3]:
                    u()
                units = units[3:]
            if i >= 1 and i < QCN - 1:
                units = _merge(units, oproj_units(i - 1))
            if i < 3:
                emit_interleaved(proj_thunks(i + 1), units)
            else:
                # final block: att(3)'s late steps are diagonal (little PE
                # work) so o_proj(2) units are back-loaded by hand; 2 are
                # held past flush(3) to cover its dependency gap.
                ou = oproj_units(i - 1)
                n_after = {5: 1, 6: 1, 7: 1, 8: 1, 9: 1, 10: 1, 11: 1,
                           12: 1, 13: 2, 14: 2, 15: 2}
                oi = 0
                for ui, u in enumerate(units):
                    u()
                    for _ in range(n_after.get(ui, 0)):
                        ou[oi]()
                        oi += 1
                while oi < len(ou):
                    ou[oi]()
                    oi += 1
        emit_interleaved([], oproj_units(3, tailq=True))

    nc.compile()
    return nc


def _merge(a, b):
    """Round-robin merge of two unit lists, proportionally."""
    out = []
    ia = ib = 0
    n = len(a) + len(b)
    for i in range(n):
        if ia * len(b) <= ib * len(a) and ia < len(a):
            out.append(a[ia]); ia += 1
        elif ib < len(b):
            out.append(b[ib]); ib += 1
        else:
            out.append(a[ia]); ia += 1
    return out


def _prep_inputs(x, Wq, Wk, Wv, Wo):
    import ml_dtypes
    bf16 = ml_dtypes.bfloat16
    xT = np.ascontiguousarray(x.reshape(L, H).T).astype(bf16)     # [H, L]
    # wave-major: xw[w][p, kc, c] = xT[kc*128+p, 512w+c]
    xW = np.ascontiguousarray(
        xT.reshape(KC, 128, LCN, 512).transpose(2, 1, 0, 3))
    cosT, sinTs = _rope_tables()
    masks = _causal_masks()
    ones = np.ones((128, 128), dtype=np.float16)
    scale = np.float32(1.0 / np.sqrt(D))
    def pmajor(w):     # [H, 256] -> [128, KC, 256] partition-major
        return np.ascontiguousarray(
            w.reshape(KC, 128, HPC * D).transpose(1, 0, 2))
    in_maps = []
    for i in range(NCORES):
        rs = slice(i * HPC * D, (i + 1) * HPC * D)
        wqk = np.ascontiguousarray(np.concatenate(
            [pmajor((Wq[rs].T * scale).astype(bf16)),
             pmajor(Wk[rs].T.astype(bf16))], axis=2))
        in_maps.append({
            "xW": xW,
            "wqkT": wqk,
            "wvT": pmajor(Wv[rs].T.astype(bf16)),
            "woP": np.ascontiguousarray(
                Wo[:, rs].T.reshape(HPC, 128, H).transpose(1, 0, 2)).astype(bf16),
            "cosT": cosT.astype(bf16),
            "sinTs": sinTs.astype(bf16),
            "masks": masks.astype(bf16),
            "ones": ones,
        })
    return in_maps


def run(x, Wq, Wk, Wv, Wo, trace=False):
    from concourse.bass_utils import run_bass_kernel_spmd
    if "nc" not in _CACHE:
        _CACHE["nc"] = _build_nc()
    nc = _CACHE["nc"]
    in_maps = _prep_inputs(np.asarray(x), np.asarray(Wq), np.asarray(Wk),
                           np.asarray(Wv), np.asarray(Wo))
    res = run_bass_kernel_spmd(nc, in_maps, core_ids=list(range(NCORES)),
                               trace=trace)
    acc = np.zeros((L, H), dtype=np.float64)
    for r in res.results:
        acc += r["out"].astype(np.float64)
    return acc.astype(np.float32).reshape(1, L, H), res


def kernel(x, Wq, Wk, Wv, Wo):
    out, _ = run(x, Wq, Wk, Wv, Wo)
    return out
